# revision 1
# baseline (speedup 1.0000x reference)
"""MultiHeadAttention (B=4, T=2048, d_model=1024, H=16, dh=64) on 8 trn2 cores.

Sharding: core c -> (batch b = c//2, head-group hg = c%2 of 8 heads).
Per-core kernel computes, feature-major throughout:
  QT = Wq_s^T x_q^T   [512, 2048]   (dq on partitions)
  KT = Wk_s^T x_k^T   [512, 2048]
  V  = x_v Wv_s       [2048, 512]   (s on partitions) + ones column per head
  ST = K_h Q_h^T      [s, t] per head; P = exp(ST/8 + mask_bias[s])
  ctxT_aug = V_aug^T P  -> [65, t]: rows 0..63 ctx^T, row 64 = softmax denom
  ctxT = ctxT_aug[0:64] * (1/denom)  broadcast over partitions via DRAM bounce
  outT_partial = Wo_s^T ctxT  [1024, 2048]
Host sums the two head-group partials per batch, transposes, adds bo.

All matmuls stream as float32r (tf32-like, full PE rate at N=512).
"""

import sys

sys.path.insert(0, "/opt/trn_rl_repo")

import numpy as np
import concourse.bass as bass
import concourse.tile as tile
from concourse import bacc, mybir
from concourse import bass_utils

B, T, DM = 4, 2048, 1024
H, DH = 16, 64
NHL = H // 2  # heads per core: 8
DQ = NHL * DH  # 512
KO = DM // 128  # 8 k-chunks over d_model
MQ = DQ // 128  # 4 M-tiles for q/k/v feature dim
NTB = T // 512  # 4 t-blocks
NS = T // 128  # 16 s-tiles
NMO = DM // 128  # 8 M-tiles for output dim
NTP = 2  # t halves for attention phase
F32 = mybir.dt.float32
F32R = mybir.dt.float32r
EXP = mybir.ActivationFunctionType.Exp

_CACHE = {}




def _build_nc():
    nc = bacc.Bacc("TRN2", target_bir_lowering=False, num_devices=8)

    xqT = nc.declare_dram_parameter("xqT", [DM, T], F32R, isOutput=False)
    xkT = nc.declare_dram_parameter("xkT", [DM, T], F32R, isOutput=False)
    xvT = nc.declare_dram_parameter("xvT", [DM, T], F32R, isOutput=False)
    wq = nc.declare_dram_parameter("wq", [DM, DQ], F32R, isOutput=False)
    wk = nc.declare_dram_parameter("wk", [DM, DQ], F32R, isOutput=False)
    wv = nc.declare_dram_parameter("wv", [DM, DQ], F32R, isOutput=False)
    wo = nc.declare_dram_parameter("wo", [DQ, DM], F32R, isOutput=False)
    bqa = nc.declare_dram_parameter("bqa", [128, MQ], F32, isOutput=False)
    bka = nc.declare_dram_parameter("bka", [128, MQ], F32, isOutput=False)
    bv = nc.declare_dram_parameter("bv", [DQ], F32, isOutput=False)
    maskb = nc.declare_dram_parameter("maskb", [128, NS], F32, isOutput=False)
    outT = nc.declare_dram_parameter("outT", [DM, T], F32, isOutput=True)

    den_dram = nc.dram_tensor("den_scratch", [NTP * NHL, 2 * 512], F32)

    with tile.TileContext(nc) as tc:
        with (
            tc.tile_pool(name="consts", bufs=1) as consts,
            tc.tile_pool(name="big", bufs=1) as big,
            tc.tile_pool(name="ps_small", bufs=4, space="PSUM") as ps_small,
            tc.tile_pool(name="ps_st", bufs=2, space="PSUM") as ps_st,
        ):
            # ---- constants ----
            bqa_sb = consts.tile([128, MQ], F32, tag="bqa")
            bka_sb = consts.tile([128, MQ], F32, tag="bka")
            maskb_sb = consts.tile([128, NS], F32, tag="maskb")
            bvb_sb = consts.tile([128, DQ], F32, tag="bvb")
            ones_sb = consts.tile([128, NHL], F32, tag="ones")
            nc.vector.memset(ones_sb[:], 1.0)
            nc.gpsimd.dma_start(out=bqa_sb[:], in_=bqa[:, :])
            nc.gpsimd.dma_start(out=bka_sb[:], in_=bka[:, :])
            nc.gpsimd.dma_start(out=maskb_sb[:], in_=maskb[:, :])
            bv_ap = bv[:]
            nc.gpsimd.dma_start(
                out=bvb_sb[:],
                in_=bass.AP(tensor=bv_ap.tensor, offset=bv_ap.offset, ap=[[0, 128]] + list(bv_ap.ap)),
            )

            # ---- big persistent tiles, split finely for dep granularity ----
            # QT[(m, tb)]: [128, 512] covering dq-tile m, t-block tb
            QT = {(m, tb): big.tile([128, 512], F32R, tag=f"QT_{m}_{tb}", name=f"QT_{m}_{tb}") for m in range(MQ) for tb in range(NTB)}
            KT = {(m, tb): big.tile([128, 512], F32R, tag=f"KT_{m}_{tb}", name=f"KT_{m}_{tb}") for m in range(MQ) for tb in range(NTB)}
            V65 = {s: big.tile([128, NHL * 65], F32R, tag=f"V65_{s}", name=f"V65_{s}") for s in range(NS)}

            # ============ Phase 1: projections ============
            with (
                tc.tile_pool(name="weights", bufs=1) as wpool,
                tc.tile_pool(name="xsl", bufs=12) as xsl_pool,
            ):
                wq_sb = wpool.tile([128, KO, DQ], F32R, tag="wq")
                wk_sb = wpool.tile([128, KO, DQ], F32R, tag="wk")
                wv_sb = wpool.tile([128, KO, DQ], F32R, tag="wv")
                wmap = {"q": (wq, wq_sb), "k": (wk, wk_sb), "v": (wv, wv_sb)}

                for tb in range(NTB):
                    tsl = slice(tb * 512, (tb + 1) * 512)
                    # Q^T and K^T : [dq, t]
                    for name, xT, w_sb, b_sb, dst in (
                        ("q", xqT, wq_sb, bqa_sb, QT),
                        ("k", xkT, wk_sb, bka_sb, KT),
                    ):
                        xs = []
                        for ko in range(KO):
                            xt = xsl_pool.tile([128, 512], F32R, tag="xsl")
                            nc.sync.dma_start(out=xt[:], in_=xT[ko * 128 : (ko + 1) * 128, tsl])
                            if tb == 0:
                                wdram, wsb = wmap[name]
                                nc.sync.dma_start(out=wsb[:, ko, :], in_=wdram[ko * 128 : (ko + 1) * 128, :])
                            xs.append(xt)
                        for m in range(MQ):
                            psum = ps_small.tile([128, 512], F32, tag="small")
                            for ko in range(KO):
                                nc.tensor.matmul(
                                    psum[:],
                                    (w_sb[:, ko, m * 128 : (m + 1) * 128]),
                                    (xs[ko][:]),
                                    start=(ko == 0),
                                    stop=(ko == KO - 1),
                                )
                            nc.vector.tensor_scalar_add(dst[(m, tb)][:], psum[:], b_sb[:, m : m + 1])
                    # V : [s, dv] in 65-strided layout with ones columns
                    xs = []
                    for ko in range(KO):
                        xt = xsl_pool.tile([128, 512], F32R, tag="xsl")
                        nc.sync.dma_start(out=xt[:], in_=xvT[ko * 128 : (ko + 1) * 128, tsl])
                        if tb == 0:
                            nc.sync.dma_start(out=wv_sb[:, ko, :], in_=wv[ko * 128 : (ko + 1) * 128, :])
                        xs.append(xt)
                    for si in range(4):
                        s = tb * 4 + si
                        psum = ps_small.tile([128, 512], F32, tag="small")
                        for ko in range(KO):
                            nc.tensor.matmul(
                                psum[:],
                                (xs[ko][:, si * 128 : (si + 1) * 128]),
                                (wv_sb[:, ko, :]),
                                start=(ko == 0),
                                stop=(ko == KO - 1),
                            )
                        v_view = V65[s][:].rearrange("p (h c) -> p h c", c=65)
                        nc.vector.tensor_copy(
                            v_view[:, :, 64:65],
                            ones_sb[:].rearrange("p (h c) -> p h c", c=1),
                        )
                        nc.vector.tensor_add(
                            v_view[:, :, 0:64],
                            psum[:].rearrange("p (h c) -> p h c", c=64),
                            bvb_sb[:].rearrange("p (h c) -> p h c", c=64),
                        )

            # ============ Phase 2+3: attention + out-proj, per t-half ============
            with (
                tc.tile_pool(name="wop", bufs=1) as wop,
                tc.tile_pool(name="ppool", bufs=6) as ppool,
                tc.tile_pool(name="ctxp", bufs=2) as ctxp,
                tc.tile_pool(name="stage", bufs=4) as stage_pool,
                tc.tile_pool(name="recip", bufs=3) as recip_pool,
                tc.tile_pool(name="recipb", bufs=3) as recipb_pool,
            ):
                wo_sb = wop.tile([128, MQ, DM], F32R, tag="wo")  # [dq, kq, dout]
                for kq in range(MQ):
                    nc.sync.dma_start(out=wo_sb[:, kq, :], in_=wo[kq * 128 : (kq + 1) * 128, :])

                ctxTs = {}

                def attn_head(tp, h):
                    q = h // 2
                    hb = (h % 2) * 64
                    ctxT_sb = ctxTs[tp]
                    ctx0 = ps_small.tile([65, 512], F32, tag="small", name=f"ctx0_{tp}_{h}")
                    ctx1 = ps_small.tile([65, 512], F32, tag="small", name=f"ctx1_{tp}_{h}")
                    ctxs = (ctx0, ctx1)
                    for s in range(NS):
                        st = ps_st.tile([128, 1024], F32, tag="st", name=f"st_{tp}_{h}_{s}")
                        for tb2 in range(2):
                            nc.tensor.matmul(
                                st[:, tb2 * 512 : (tb2 + 1) * 512],
                                (KT[(q, s // 4)][hb : hb + 64, (s % 4) * 128 : (s % 4 + 1) * 128]),
                                (QT[(q, tp * 2 + tb2)][hb : hb + 64, :]),
                                start=True,
                                stop=True,
                            )
                        p_sb = ppool.tile([128, 1024], F32R, tag="p", name=f"p_{tp}_{h}_{s}")
                        nc.scalar.activation(
                            out=p_sb[:],
                            in_=st[:],
                            func=EXP,
                            bias=maskb_sb[:, s : s + 1],
                            scale=0.125,
                        )
                        for tb2 in range(2):
                            nc.tensor.matmul(
                                ctxs[tb2][:],
                                (V65[s][:, h * 65 : (h + 1) * 65]),
                                (p_sb[:, tb2 * 512 : (tb2 + 1) * 512]),
                                start=(s == 0),
                                stop=(s == NS - 1),
                            )
                    # softmax denominator -> reciprocal -> partition-broadcast
                    rc = recip_pool.tile([1, 2 * 512], F32, tag="recip", name=f"rc_{tp}_{h}")
                    for tb2 in range(2):
                        nc.vector.reciprocal(
                            out=rc[0:1, tb2 * 512 : (tb2 + 1) * 512],
                            in_=ctxs[tb2][64:65, :],
                        )
                    idx = tp * NHL + h
                    nc.gpsimd.dma_start(out=den_dram[idx : idx + 1, :], in_=rc[:])
                    rb = recipb_pool.tile([64, 2, 512], F32, tag="recipb", name=f"rb_{tp}_{h}")
                    dd = den_dram[idx, :]
                    nc.gpsimd.dma_start(
                        out=rb[:],
                        in_=bass.AP(
                            tensor=dd.tensor,
                            offset=dd.offset,
                            ap=[[0, 64], [512, 2], [1, 512]],
                        ),
                    )
                    for tb2 in range(2):
                        nc.vector.tensor_mul(
                            ctxT_sb[hb : hb + 64, q, tb2 * 512 : (tb2 + 1) * 512],
                            ctxs[tb2][0:64, :],
                            rb[:, tb2, :],
                        )

                def op_chain(tp, mo, tb2):
                    ctxT_sb = ctxTs[tp]
                    psum = ps_small.tile(
                        [128, 512], F32, tag="small", name=f"op_{tp}_{mo}_{tb2}"
                    )
                    for kq in range(MQ):
                        nc.tensor.matmul(
                            psum[:],
                            (wo_sb[:, kq, mo * 128 : (mo + 1) * 128]),
                            (ctxT_sb[:, kq, tb2 * 512 : (tb2 + 1) * 512]),
                            start=(kq == 0),
                            stop=(kq == MQ - 1),
                        )
                    stg = stage_pool.tile([128, 512], F32, tag="stage", name=f"stg_{tp}_{mo}_{tb2}")
                    nc.vector.tensor_copy(stg[:], psum[:])
                    nc.sync.dma_start(
                        out=outT[
                            mo * 128 : (mo + 1) * 128,
                            tp * 1024 + tb2 * 512 : tp * 1024 + (tb2 + 1) * 512,
                        ],
                        in_=stg[:],
                    )

                ctxTs[0] = ctxp.tile([128, MQ, 1024], F32R, tag="ctxT", name="ctxT_0")
                for h in range(NHL):
                    attn_head(0, h)
                ctxTs[1] = ctxp.tile([128, MQ, 1024], F32R, tag="ctxT", name="ctxT_1")
                op0 = [(0, mo, tb2) for mo in range(NMO) for tb2 in range(2)]
                for h in range(NHL):
                    attn_head(1, h)
                    for _ in range(2):
                        if op0:
                            op_chain(*op0.pop(0))
                for args in op0:
                    op_chain(*args)
                for mo in range(NMO):
                    for tb2 in range(2):
                        op_chain(1, mo, tb2)

    nc.finalize()
    return nc


def _get_nc():
    if "nc" not in _CACHE:
        _CACHE["nc"] = _build_nc()
    return _CACHE["nc"]


def _get_runner():
    """Persistent jitted 8-core runner (compiles the NEFF once per process)."""
    if "runner" in _CACHE:
        return _CACHE["runner"]
    import jax
    from jax.experimental.shard_map import shard_map
    from jax.sharding import Mesh, PartitionSpec
    from concourse import bass2jax

    nc = _get_nc()
    bass2jax.install_neuronx_cc_hook()
    partition_name = nc.partition_id_tensor.name if nc.partition_id_tensor else None
    in_names, out_names, out_avals = [], [], []
    for alloc in nc.m.functions[0].allocations:
        if not isinstance(alloc, mybir.MemoryLocationSet):
            continue
        name = alloc.memorylocations[0].name
        if alloc.kind == "ExternalInput":
            if name != partition_name:
                in_names.append(name)
        elif alloc.kind == "ExternalOutput":
            out_names.append(name)
            out_avals.append(
                jax.core.ShapedArray(tuple(alloc.tensor_shape), mybir.dt.np(alloc.dtype))
            )
    n_params = len(in_names)
    n_outs = len(out_avals)
    all_names = list(in_names) + list(out_names)
    if partition_name is not None:
        all_names.append(partition_name)

    def _body(*args):
        operands = list(args)
        if partition_name is not None:
            operands.append(bass2jax.partition_id_tensor())
        outs = bass2jax._bass_exec_p.bind(
            *operands,
            out_avals=tuple(out_avals),
            in_names=tuple(all_names),
            out_names=tuple(out_names),
            lowering_input_output_aliases=(),
            sim_require_finite=True,
            sim_require_nnan=True,
            nc=nc,
        )
        return tuple(outs)

    devices = jax.devices()[:8]
    mesh = Mesh(np.asarray(devices), ("core",))
    in_specs = (PartitionSpec("core"),) * (n_params + n_outs)
    out_specs = (PartitionSpec("core"),) * n_outs
    fn = jax.jit(
        shard_map(_body, mesh=mesh, in_specs=in_specs, out_specs=out_specs, check_rep=False),
        donate_argnums=tuple(range(n_params, n_params + n_outs)),
        keep_unused=True,
    )
    runner = (fn, list(in_names), list(out_names), list(out_avals))
    _CACHE["runner"] = runner
    return runner


def _run_cores(in_maps):
    """Run the SPMD kernel on 8 cores; returns list of per-core output dicts."""
    fn, in_names, out_names, out_avals = _get_runner()
    concat_in = [
        np.concatenate([np.asarray(m[name]) for m in in_maps], axis=0) for name in in_names
    ]
    concat_zeros = [
        np.zeros((8 * a.shape[0], *a.shape[1:]), a.dtype) for a in out_avals
    ]
    out_arrs = fn(*concat_in, *concat_zeros)
    return [
        {
            name: np.asarray(out_arrs[i]).reshape(8, *out_avals[i].shape)[c]
            for i, name in enumerate(out_names)
        }
        for c in range(8)
    ]


def kernel(
    x_Q, x_K, x_V, Wq, bq, Wk, bk, Wv, bv, Wo, bo, src_batch_lens, **_unused
):
    x_Q = np.ascontiguousarray(np.asarray(x_Q, dtype=np.float32))
    x_K = np.ascontiguousarray(np.asarray(x_K, dtype=np.float32))
    x_V = np.ascontiguousarray(np.asarray(x_V, dtype=np.float32))
    Wq = np.asarray(Wq, dtype=np.float32)
    Wk = np.asarray(Wk, dtype=np.float32)
    Wv = np.asarray(Wv, dtype=np.float32)
    Wo = np.asarray(Wo, dtype=np.float32)
    bq = np.asarray(bq, dtype=np.float32)
    bk = np.asarray(bk, dtype=np.float32)
    bv = np.asarray(bv, dtype=np.float32)
    bo = np.asarray(bo, dtype=np.float32)
    lens = np.asarray(src_batch_lens).astype(np.int64)

    in_maps = []
    s_idx = np.arange(T)
    for c in range(8):
        b, hg = c // 2, c % 2
        cols = slice(hg * DQ, (hg + 1) * DQ)
        mask_bias = np.where(s_idx < lens[b], 0.0, -1e9).astype(np.float32)
        in_maps.append(
            {
                "xqT": np.ascontiguousarray(x_Q[b].T),
                "xkT": np.ascontiguousarray(x_K[b].T),
                "xvT": np.ascontiguousarray(x_V[b].T),
                "wq": np.ascontiguousarray(Wq[:, cols]),
                "wk": np.ascontiguousarray(Wk[:, cols]),
                "wv": np.ascontiguousarray(Wv[:, cols]),
                "wo": np.ascontiguousarray(Wo[cols, :]),
                "bqa": np.ascontiguousarray(bq[cols].reshape(MQ, 128).T),
                "bka": np.ascontiguousarray(bk[cols].reshape(MQ, 128).T),
                "bv": np.ascontiguousarray(bv[cols]),
                "maskb": np.ascontiguousarray(mask_bias.reshape(NS, 128).T),
            }
        )

    try:
        res = _run_cores(in_maps)
    except Exception:
        nc = _get_nc()
        res = bass_utils.run_bass_kernel_spmd(nc, in_maps, list(range(8))).results

    out = np.empty((B, T, DM), dtype=np.float32)
    for b in range(B):
        acc = res[2 * b]["outT"] + res[2 * b + 1]["outT"]
        out[b] = acc.T + bo[None, :]
    return out

